# revision 18
# baseline (speedup 1.0000x reference)
"""Conv2D 3x3 (stride 1, pad 1) Trainium2 Bass kernel.

Problem: x (32, 64, 64, 64) NCHW fp32, weight (128, 64, 3, 3) OIHW, bias (128,).
Output: (32, 128, 64, 64).

Strategy: data-parallel over batch across 8 cores (4 images/core), bf16
matmuls (tolerance is 2e-2; bf16 with fp32 PSUM accumulation lands ~3e-3
and streams the PE at 1 col/cycle vs ~3 for fp32r). The host pre-pads each
image channel into a 66x66 zero-ringed layout (+ tail slack) in bf16.
On-chip, partitions 0-63 hold the padded channels and partitions 64-127
hold the same data shifted down one padded row (a second chunked DMA of
the same HBM bytes at offset 66), so a single K=128 matmul contracts two
kernel-row taps at once. Conv per 396-pixel PSUM tile = 3 paired K=128
matmuls (ky={0,1} x kx) + 3 K=64 matmuls for ky=2, two of which are
row-tiled to opposite PE halves so they stream concurrently -> 5 serial
streaming slots per tile. x DMAs are chunked so image-0 compute starts
after ~1/4 of the data. A dep-free warm-up matmul chain keeps the PE HAM
clock-gate at 2.4 GHz before real work lands. Bias-add fuses into the
PSUM->SBUF eviction on the scalar engine; output stores as bf16 and is
upcast on the host.
"""

import numpy as np
import ml_dtypes

import concourse.bass as bass
import concourse.mybir as mybir
import concourse.tile as tile
from concourse import bacc
from concourse.bass_utils import run_bass_kernel_spmd

N_CORES = 8
NIMG = 4  # images per core
C = 64  # input channels
H = W = 64
O = 128  # output channels
PW = 66  # padded row length
PH = 66  # padded rows
IMG = PH * PW  # 4356 padded elements per channel per image
# Row-aligned PSUM tiles: 4 groups of 7 output rows + 6 of 6 rows.
# 7*66 = 462 fp32 still fits a 2 KiB PSUM bank; fewer tiles = fewer
# per-tile overheads (group-start matmul, evictions).
TILE_ROWS = [7] * 4 + [6] * 6
NQT = len(TILE_ROWS)  # 10
MAXR = max(TILE_ROWS)
# Lower copy: max read = 60*66 + 134 + 264 = 4358. Upper copy (shifted by
# one padded row): max read = 60*66 + 67 + 264 = 4291.
XCOLS = 4360
UPLEN = 4292
NCHUNK = 4  # x DMA chunks per copy per image
NWARM = 12  # dep-free warm-up matmuls to spin the HAM clock-gate up

import os
NWARM = int(os.environ.get("KWARM", NWARM))
ROWTILE = int(os.environ.get("KROWTILE", "3"))

F32 = mybir.dt.float32
BF16 = mybir.dt.bfloat16

_CACHED_NC = None


def build_nc():
    nc = bacc.Bacc()
    x_in = nc.declare_dram_parameter("xp", [NIMG, C, XCOLS], BF16, isOutput=False)
    w_in = nc.declare_dram_parameter("wcat", [2 * C, 6, O], BF16, isOutput=False)
    b_in = nc.declare_dram_parameter("bias", [O, 1], F32, isOutput=False)
    out = nc.declare_dram_parameter("out", [NIMG, O, H, W], BF16, isOutput=True)

    with tile.TileContext(nc) as tc:
        with (
            tc.tile_pool(name="const", bufs=1) as const_pool,
            tc.tile_pool(name="xp", bufs=3) as x_pool,
            tc.tile_pool(name="osb", bufs=3) as o_pool,
            tc.tile_pool(
                name="psum", bufs=(6 if ROWTILE == 3 else 7), space="PSUM"
            ) as psum_pool,
            tc.tile_pool(name="aux", bufs=2, space="PSUM") as aux_pool,
            tc.tile_pool(name="tmp", bufs=4) as tmp_pool,
        ):
            # Warm-up chain: memset a scratch tile (no DMA dependency), then
            # issue back-to-back dummy matmuls. These run while the runtime
            # is still fetching instructions / loading the first x chunks,
            # so the HAM clock-gate is at K=8/8 (2.4 GHz) when real matmuls
            # start. They must finish before the first x chunk lands, so
            # they cost nothing if NWARM is sized right.
            if NWARM:
                scratch = const_pool.tile([128, 448], BF16)
                nc.gpsimd.memset(scratch[:, :], 0.0)
                warm = aux_pool.tile([128, MAXR * PW], F32, tag="acc")
                for _ in range(NWARM):
                    nc.tensor.matmul(
                        warm[:, 0:448], scratch[:, 0:128], scratch[:, 0:448],
                        start=True, stop=True,
                    )

            wcat = const_pool.tile([2 * C, 6, O], BF16)
            bias_t = const_pool.tile([O, 1], F32)
            nc.sync.dma_start(wcat[:, :, :], w_in[:, :, :])
            nc.sync.dma_start(bias_t[:, :], b_in[:, :])

            lb = [XCOLS * k // NCHUNK for k in range(NCHUNK + 1)]
            ub = [UPLEN * k // NCHUNK for k in range(NCHUNK + 1)]
            for m in range(NIMG):
                xt = x_pool.tile([128, XCOLS], BF16)
                # lower half: padded image; upper half: same shifted one
                # padded row (pairs kernel rows ky=0/1 in one K=128 matmul).
                # Chunked + interleaved so tile 0's matmuls only wait for
                # the first ~1/NCHUNK of the image.
                for k in range(NCHUNK):
                    nc.sync.dma_start(
                        xt[0:C, lb[k] : lb[k + 1]], x_in[m, :, lb[k] : lb[k + 1]]
                    )
                    nc.sync.dma_start(
                        xt[C : 2 * C, ub[k] : ub[k + 1]],
                        x_in[m, :, PW + ub[k] : PW + ub[k + 1]],
                    )

                osb = o_pool.tile([O, H * W], BF16)
                r0 = 0
                for t in range(NQT):
                    rows = TILE_ROWS[t]
                    q0 = r0 * PW
                    qt = rows * PW
                    acc = psum_pool.tile([O, MAXR * PW], F32, tag="acc")
                    # Accumulation group per tile, as (weight-slot-AP, x-AP)
                    # pairs. The group must OPEN and CLOSE with full K=128
                    # matmuls: a row-tiled K=64 matmul on the upper PE half
                    # runs concurrently with lower-half K=64 neighbors, and
                    # only a full-array matmul is guaranteed to complete
                    # after it, so the completion semaphore (which gates the
                    # scalar eviction of this PSUM bank) must hang off a
                    # K=128 matmul or the eviction races the drain (fatal
                    # PSUM collision).
                    lo, hi, full = slice(0, C), slice(C, 2 * C), slice(0, 2 * C)
                    pair = lambda kx: (full, kx, q0 + kx)  # ky0+ky1 paired
                    if ROWTILE == 3:
                        # 5-slot schedule: the ky2/kx1 tap runs on PE rows
                        # 64-127 into its OWN PSUM bank (concurrent drains
                        # into one bank are fatal), overlapping the two
                        # lower-half ky2 matmuls. The scalar engine evicts
                        # the aux bank (+bias) into an SBUF tmp while the
                        # closing pairs still stream; the vector engine then
                        # fuses main+tmp -> bf16 osb.
                        aux = aux_pool.tile([O, MAXR * PW], F32, tag="acc")
                        # open the group on a K=64 (a full->K64-subset
                        # boundary chains cheaply); close on a full pair so
                        # the completion semaphore is ordered after the
                        # concurrent aux matmul
                        nc.tensor.matmul(
                            acc[:, 0:qt], wcat[lo, 3, :],
                            xt[lo, q0 + 132 : q0 + 132 + qt],
                            start=True, stop=False,
                        )
                        nc.tensor.matmul(
                            aux[:, 0:qt], wcat[hi, 3, :],
                            xt[hi, q0 + 67 : q0 + 67 + qt],
                            start=True, stop=True,
                        )
                        nc.tensor.matmul(
                            acc[:, 0:qt], wcat[lo, 4, :],
                            xt[lo, q0 + 134 : q0 + 134 + qt],
                            start=False, stop=False,
                        )
                        for kx in (1, 2, 0):
                            nc.tensor.matmul(
                                acc[:, 0:qt], wcat[full, kx, :],
                                xt[full, q0 + kx : q0 + kx + qt],
                                start=False, stop=(kx == 0),
                            )
                        tmpt = tmp_pool.tile([O, MAXR * W], F32)
                        axv = aux[:, 0:qt].rearrange("p (r c) -> p r c", c=PW)
                        tmv = tmpt[:, 0 : rows * W].rearrange(
                            "p (r c) -> p r c", c=W
                        )
                        nc.scalar.activation(
                            tmv[:, :, :],
                            axv[:, :, 0:W],
                            mybir.ActivationFunctionType.Identity,
                            bias=bias_t[:, :],
                        )
                        av = acc[:, 0:qt].rearrange("p (r c) -> p r c", c=PW)
                        ov = osb[:, r0 * W : (r0 + rows) * W].rearrange(
                            "p (r c) -> p r c", c=W
                        )
                        nc.vector.tensor_tensor(
                            ov[:, :, :],
                            av[:, :, 0:W],
                            tmv[:, :, :],
                            mybir.AluOpType.add,
                        )
                        r0 += rows
                        continue
                    if ROWTILE == 1:
                        mms = [
                            pair(0),
                            (lo, 3, q0 + 132),   # ky2 kx0, lower rows
                            (hi, 3, q0 + 67),    # ky2 kx1, upper rows (concurrent)
                            (lo, 4, q0 + 134),   # ky2 kx2, lower rows
                            pair(1),
                            pair(2),
                        ]
                    elif ROWTILE == 2:
                        # diagnostic: upper-rows matmul sandwiched between
                        # full K=128 matmuls -> no concurrency anywhere
                        mms = [
                            pair(0),
                            (hi, 3, q0 + 67),
                            pair(1),
                            (lo, 3, q0 + 132),
                            (lo, 4, q0 + 134),
                            pair(2),
                        ]
                    else:
                        mms = [
                            pair(0),
                            (lo, 3, q0 + 132),
                            (lo, 5, q0 + 133),
                            (lo, 4, q0 + 134),
                            pair(1),
                            pair(2),
                        ]
                    for i, (part, slot, off) in enumerate(mms):
                        nc.tensor.matmul(
                            acc[:, 0:qt],
                            wcat[part, slot, :],
                            xt[part, off : off + qt],
                            start=(i == 0),
                            stop=(i == len(mms) - 1),
                        )
                    # evict + bias add on the scalar engine, dropping the 2
                    # garbage columns per row so osb is contiguous valid data
                    av = acc[:, 0:qt].rearrange("p (r c) -> p r c", c=PW)
                    ov = osb[:, r0 * W : (r0 + rows) * W].rearrange(
                        "p (r c) -> p r c", c=W
                    )
                    nc.scalar.activation(
                        ov[:, :, :],
                        av[:, :, 0:W],
                        mybir.ActivationFunctionType.Identity,
                        bias=bias_t[:, :],
                    )
                    r0 += rows

                # contiguous stores on the ACT HWDGE queue, split so earlier
                # rows fly while later tiles still compute and the final
                # exposed store is only a quarter image
                store_eng = nc.scalar if m < 2 else nc.sync
                for a, b in ((0, 32), (32, 48), (48, 56), (56, 64)):
                    store_eng.dma_start(
                        out[m, :, a:b, :], osb[:, a * W : b * W]
                    )

    nc.compile()
    return nc


def _prep_inputs(x, weight, bias):
    bf16 = ml_dtypes.bfloat16
    x = np.asarray(x, dtype=np.float32)
    n = x.shape[0]
    z = np.zeros((n, C, PH, PW), dtype=bf16)
    z[:, :, 1 : 1 + H, 1 : 1 + W] = x.astype(bf16)
    xp = np.zeros((n, C, XCOLS), dtype=bf16)
    xp[:, :, :IMG] = z.reshape(n, C, IMG)

    w_t = np.asarray(weight, dtype=np.float32).astype(bf16).transpose(1, 2, 3, 0)
    wcat = np.zeros((2 * C, 6, O), dtype=bf16)
    wcat[0:C, 0:3, :] = w_t[:, 0, :, :]  # ky=0 (lower half of pairs)
    wcat[C : 2 * C, 0:3, :] = w_t[:, 1, :, :]  # ky=1 (upper half of pairs)
    wcat[0:C, 3, :] = w_t[:, 2, 0, :]  # ky=2 kx=0 (lower rows)
    wcat[C : 2 * C, 3, :] = w_t[:, 2, 1, :]  # ky=2 kx=1 (upper rows)
    wcat[0:C, 4, :] = w_t[:, 2, 2, :]  # ky=2 kx=2 (lower rows)
    wcat[0:C, 5, :] = w_t[:, 2, 1, :]  # ky=2 kx=1 (lower rows, non-rowtile fallback)
    b = np.ascontiguousarray(np.asarray(bias, dtype=np.float32).reshape(O, 1))
    return xp, wcat, b


def _in_maps(x, weight, bias):
    xp, wcat, b = _prep_inputs(x, weight, bias)
    return [
        {"xp": xp[i * NIMG : (i + 1) * NIMG], "wcat": wcat, "bias": b}
        for i in range(N_CORES)
    ]


def kernel(x: np.ndarray, weight: np.ndarray, bias: np.ndarray) -> np.ndarray:
    global _CACHED_NC
    if _CACHED_NC is None:
        _CACHED_NC = build_nc()
    res = run_bass_kernel_spmd(_CACHED_NC, _in_maps(x, weight, bias), list(range(N_CORES)))
    return np.concatenate([r["out"] for r in res.results], axis=0).astype(np.float32)


def run_profiled(x, weight, bias, tmpdir=None):
    """Dev helper: run with NTFF tracing, return BassKernelResults."""
    global _CACHED_NC
    if _CACHED_NC is None:
        _CACHED_NC = build_nc()
    return run_bass_kernel_spmd(
        _CACHED_NC, _in_maps(x, weight, bias), list(range(N_CORES)),
        trace=True, tmpdir=tmpdir,
    )
